# revision 34
# baseline (speedup 1.0000x reference)
"""Bass/Trainium2 kernel for masked dot-product attention.

Math (per batch b):
  scores = q @ k^T / sqrt(D)
  masked positions (j >= valid[i]) replaced by 1e-6 (NOT -inf)
  weights = softmax(scores, axis=-1);  out = weights @ v

Strategy:
  - Shard batch dim B=16 across 8 cores (2 batches/core), SPMD program.
  - Host-side: sort rows of each batch by valid[i] (argsort) so the mask is a
    monotone staircase; device computes only the staircase-covered region.
  - Device per (batch, 512-row i-range):
      S^T tiles [j=128, i<=512] on PE (fp16 operands),
      exp split row-consistently between ACT (spline exp, rows < cut) and DVE
      (rows >= cut; one-instruction fp16 "Blinn" bit-trick exp:
      bits = round(s*1024*log2e/8 + offset) written as int16 aliasing the fp16
      e-tile; offset centers the PWL-mantissa log-error at +-3%, and its known
      mean bias is corrected exactly on the host),
      boundary-tile masking via one fused scalar_tensor_tensor (is_lt -> mult)
      load-balanced between DVE and GpSimd,
      PV accumulated per 128-row i-subblock as pacc[i, 0:65] += E_tile^T.T @ V
      (output free dim 65 instead of 512 -> ~2x fewer PE cycles), ones column
      in V gives the softmax denominator for free. PSUM zero regions are one
      whole 2KB bank, so the range's pacc bank gets exactly one start (zeroes
      the bank) and one stop.
  - Out: pacc (PSUM f32) scaled by 1/16 into fp16 SBUF (ACT/DVE alternating),
    DMA'd out on the sync HWDGE ring after all input loads.
  - Host: adds the analytic masked-region correction exp(1e-6)*(suffix sums of
    v) (scaled by 1+BIAS for Blinn rows), divides by the denominator, fills
    never-written rows, unsorts.
"""

import numpy as np

import concourse.bass as bass
import concourse.tile as tile
import concourse.mybir as mybir
from concourse import bacc
from concourse.bass_utils import run_bass_kernel_spmd

B, N, D = 16, 2048, 64
NCORES = 8
NB = B // NCORES          # batches per core
IW = 512                  # i-range width (moving dim of S matmuls)
NI = N // IW              # 4 i-ranges
JW = 128                  # j-tile width (partition dim of S^T)
NJ = N // JW              # 16 j-tiles
DV = D + 1                # V with ones column appended
G = 2                     # j-tiles per exp group (PSUM: 3*2 + 2 = 8 banks)

f32 = mybir.dt.float32
f16 = mybir.dt.float16
i16 = mybir.dt.int16
i32 = mybir.dt.int32

SCALE = 0.125             # 1/sqrt(D)
# Blinn fp16 exp: bits = round(s*BL_MUL + BL_OFF) viewed as fp16 ~= e^(s/8).
# g(f) = log2(1+f) - f in [0, 0.08607]; centering at c = 0.04304 makes the
# multiplicative noise +-3.03% with mean bias E[2^(g-c)] = 1 + BL_BIAS that
# the host corrects exactly.
_C_CENTER = 0.0430374
BL_MUL = float(1024.0 * SCALE / np.log(2.0))
BL_OFF = float(15360.0 - 1024.0 * _C_CENTER)
_f = np.linspace(0.0, 1.0, 200001)
BL_BIAS = float(np.trapezoid((1.0 + _f) * 2.0 ** (-_f), _f) * 2.0 ** (-_C_CENTER) - 1.0)
OSCALE = 1.0 / 16.0       # pacc -> fp16 out scaling (overflow headroom)

# cost-model constants for the ACT/DVE/Pool balance (ns per element / instr)
_ACT_NS = 0.8333
_DVE_NS = 1.0417
_POOL_NS = 0.8333 / 0.6
_ACT_OH = 185.0
_DVE_OH = 125.0
_POOL_OH = 160.0


def _round_pv(alo):
    m = alo % 128
    return (alo // 128) * 128 + (64 if m >= 64 else (32 if m >= 32 else 0))


def _classify(t_sorted):
    """Per (i-range, j-tile): None (skip) or (pvlo, mhi): compute columns
    [pvlo, IW) (pvlo rounded down so partition bases land on {0,32,64});
    the fused mask zeroes [pvlo, mhi)."""
    cls = []
    for r in range(NI):
        row = []
        tw = t_sorted[:, r * IW:(r + 1) * IW]  # [B, IW] sorted ascending
        for tau in range(NJ):
            jlo, jhi = JW * tau, JW * (tau + 1)
            n_le = (tw <= jlo).sum(axis=1)   # rows fully masked in this tile
            n_lt = (tw < jhi).sum(axis=1)    # rows with any mask in this tile
            if (n_le == IW).all():
                row.append(None)
            else:
                alo = int(n_le.min())
                pvlo = _round_pv(alo)
                mhi = max(int(n_lt.max()), alo)
                row.append((pvlo, max(mhi, pvlo)))
        cls.append(row)
    return cls


def _groups(cls, r):
    taus = [t for t in range(NJ) if cls[r][t] is not None]
    return taus, [taus[i:i + G] for i in range(0, len(taus), G)]


def _plan(cls):
    """Numerically balance ACT / DVE / Pool exp shares.

    Rows [plo, cut1) -> ACT spline exp; [cut1, cut2) -> DVE Blinn;
    [cut2, IW) -> Pool Blinn. Returns (cut1, cut2) per range."""
    ranges = []
    masks = []
    conv_n = 0
    for r in range(NI):
        taus, groups = _groups(cls, r)
        if taus:
            conv_n += 1
        plos = [min(cls[r][t][0] for t in gt) for gt in groups]
        nhs = [len(gt) for gt in groups]
        ranges.append((plos, nhs))
        for t in taus:
            pvlo, mhi = cls[r][t]
            if mhi > pvlo:
                masks.append((r, t, mhi - pvlo))
    conv_n *= NB

    def predict(f_act, g_pool):
        cut1 = []
        act = dve = pool = 0.0
        for plos, nhs in ranges:
            if not plos:
                cut1.append(IW)
                continue
            area = sum((IW - p) * n for p, n in zip(plos, nhs))

            def pick(frac_above):
                best, bestd = IW, abs(frac_above * area)
                for c in range(0, IW + 1, 16):
                    above = sum((IW - max(c, p)) * n for p, n in zip(plos, nhs))
                    dd = abs(above - frac_above * area)
                    if dd < bestd:
                        best, bestd = c, dd
                return best

            c1 = pick(1.0 - f_act)
            cut1.append(c1)
            for p, n in zip(plos, nhs):
                aw = max(0, min(c1, IW) - p)
                dw = IW - max(c1, p)
                if aw > 0:
                    act += (_ACT_NS * aw * n + _ACT_OH) * NB
                if dw > 0:
                    dve += (_DVE_NS * dw * n + _DVE_OH) * NB
        # offload the mult half of the g_pool largest mask columns to Pool
        eng = {}
        order = sorted(masks, key=lambda m: -m[2])
        target = sum(m[2] for m in masks) * g_pool
        acc = 0.0
        for r_, t_, w in order:
            if acc < target:
                eng[(r_, t_)] = 'p'
                dve += (0.52 * w + _DVE_OH) * NB          # is_lt at 2x
                pool += (2.0 * w + 255.0) * NB            # mult, eff 0.42
                acc += w
            else:
                eng[(r_, t_)] = 'd'
                dve += (_DVE_NS * w + _DVE_OH) * NB       # fused STT
        act += (conv_n // 2 + conv_n % 2) * (_ACT_NS * 4 * DV + _ACT_OH)
        dve += (conv_n // 2) * (_DVE_NS * 4 * DV + _DVE_OH)
        act += 1283.0   # act table load
        return act, dve, pool, cut1, eng

    # knobs fitted against TimelineSim sweeps (predict() underestimates the
    # critical-path effects, so the analytic argmin is not used directly)
    _, _, _, cut1, eng = predict(0.63, 0.4)
    return cut1, eng


LAST_BUILD_OPTS = {}


def _build_program(cls, cut1, mask_eng, d_e=2, d_t=4):
    nc = bacc.Bacc("TRN2", target_bir_lowering=False, debug=False)

    qT = nc.dram_tensor("qT", [NB, D, N], f16, kind="ExternalInput").ap()
    kT = nc.dram_tensor("kT", [NB, D, N], f16, kind="ExternalInput").ap()
    vw = nc.dram_tensor("vw", [NB, 128, NJ * DV], f16, kind="ExternalInput").ap()
    tbnd = nc.dram_tensor("tbnd", [NB, 1, N], f16, kind="ExternalInput").ap()
    out = nc.dram_tensor("out", [NB, NI, 128, 4 * DV], f16,
                         kind="ExternalOutput").ap()

    H = N // 2

    with tile.TileContext(nc, trace_sim=False) as tc:
        with (
            tc.tile_pool(name="consts", bufs=1) as consts,
            tc.tile_pool(name="sb_T", bufs=2) as sb_T,
            tc.tile_pool(name="sb_v", bufs=2) as sb_v,
            tc.tile_pool(name="sb_e", bufs=6) as sb_e,
            tc.tile_pool(name="sb_o", bufs=3) as sb_o,
            tc.tile_pool(name="sb_m", bufs=6) as sb_m,
            tc.tile_pool(name="ps_s", bufs=3, space="PSUM") as ps_s,
            tc.tile_pool(name="ps_acc", bufs=2, space="PSUM") as ps_acc,
        ):
            jpos_i = consts.tile([128, NJ], i32)
            nc.gpsimd.iota(jpos_i, pattern=[[-JW, NJ]], base=1024,
                           channel_multiplier=-1)
            jpos = consts.tile([128, NJ], f32)
            nc.vector.tensor_copy(jpos, jpos_i)

            tiles = []
            tbms = []
            for bi in range(NB):
                qTs = sb_T.tile([D, N], f16, tag="qT", name=f"qT{bi}")
                kTs = sb_T.tile([D, N], f16, tag="kT", name=f"kT{bi}")
                tbn = sb_T.tile([128, N], f16, tag="tbn", name=f"tbn{bi}")
                vws = sb_v.tile([128, NJ, DV], f16, tag="vw", name=f"vw{bi}")
                # need-ordered chunked loads on three HWDGE rings
                nc.sync.dma_start(out=kTs[:, 0:H], in_=kT[bi][:, 0:H])
                nc.scalar.dma_start(out=qTs[:, 0:H], in_=qT[bi][:, 0:H])
                nc.sync.dma_start(out=kTs[:, H:N], in_=kT[bi][:, H:N])
                nc.scalar.dma_start(out=qTs[:, H:N], in_=qT[bi][:, H:N])
                nc.sync.dma_start(out=vws, in_=vw[bi])
                tbm = sb_v.tile([1, N], f16, tag="tbm", name=f"tbm{bi}")
                nc.gpsimd.dma_start(out=tbm, in_=tbnd[bi])
                if bi == 0:
                    nc.gpsimd.partition_broadcast(tbn[:, 0:IW], tbm[:, 0:IW])
                    nc.gpsimd.partition_broadcast(tbn[:, IW:H], tbm[:, IW:H])
                    nc.gpsimd.partition_broadcast(tbn[:, H:N], tbm[:, H:N])
                tbms.append(tbm)
                tiles.append((qTs, kTs, tbn, vws))

            # flatten all (batch, range, group) work; software-pipeline three
            # stages (S at i+2, exp at i+1, masks+PV at i) so waiting PV
            # matmuls never head-of-line-block the next S on the PE queue.
            per_bi = []
            for bi in range(NB):
                wl = []
                rs = range(NI) if bi == 0 else range(NI - 1, -1, -1)
                for r in rs:
                    taus, groups = _groups(cls, r)
                    if not taus:
                        continue
                    pv_seq = [(t, ib) for t in taus
                              for ib in range(cls[r][t][0] // 128, 4)]
                    for g, gt in enumerate(groups):
                        wl.append((bi, r, gt, g == 0, g == len(groups) - 1,
                                   pv_seq[0], pv_seq[-1]))
                per_bi.append(wl)
            if LAST_BUILD_OPTS.get("interleave"):
                work = []
                a, b = per_bi
                for i in range(max(len(a), len(b))):
                    if i < len(a):
                        work.append(a[i])
                    if i < len(b):
                        work.append(b[i])
            else:
                work = per_bi[0] + per_bi[1]

            conv_i = 0
            st = {}      # per in-flight index: (ps, e, pacc_of_range)
            pacc_cur = None

            bcast_done = [False]

            def stage_s(i):
                nonlocal pacc_cur
                bi, r, gt, fst, lst, pvf, pvl = work[i]
                qTs, kTs, tbn, vws = tiles[bi]
                if bi == 1 and not bcast_done[0]:
                    bcast_done[0] = True
                    tb1 = tiles[1][2]
                    nc.gpsimd.partition_broadcast(tb1[:, 0:H], tbms[1][:, 0:H])
                    nc.gpsimd.partition_broadcast(tb1[:, H:N], tbms[1][:, H:N])
                nh = len(gt)
                ps = ps_s.tile([128, G, IW], f32)
                for h, t in enumerate(gt):
                    pvlo = cls[r][t][0]
                    nc.tensor.matmul(
                        ps[:, h, pvlo:IW],
                        kTs[:, bass.ts(t, JW)],
                        qTs[:, r * IW + pvlo:(r + 1) * IW],
                        start=True, stop=True)
                ms = {}
                for h, t in enumerate(gt):
                    pvlo, mhi = cls[r][t]
                    if mhi > pvlo and mask_eng.get((r, t)) == 'p':
                        m = sb_m.tile([128, IW], f16, tag="m",
                                      name=f"m{i}_{h}")
                        ieng = (nc.gpsimd if LAST_BUILD_OPTS.get("pool_islt")
                                else nc.vector)
                        ieng.tensor_scalar(
                            out=m[:, 0:mhi - pvlo],
                            in0=tbn[:, r * IW + pvlo:r * IW + mhi],
                            scalar1=jpos[:, t:t + 1], scalar2=None,
                            op0=mybir.AluOpType.is_lt)
                        ms[h] = m
                st[i] = [ps, None, ms]

            def stage_e(i):
                bi, r, gt, fst, lst, pvf, pvl = work[i]
                nh = len(gt)
                ps = st[i][0]
                e = sb_e.tile([128, G, IW], f16)
                plo = min(cls[r][t][0] for t in gt)
                cs1 = min(max(cut1[r], plo), IW)
                if cs1 > plo:
                    nc.scalar.activation(
                        e[:, 0:nh, plo:cs1], ps[:, 0:nh, plo:cs1],
                        mybir.ActivationFunctionType.Exp, scale=SCALE)
                if cs1 < IW:
                    nc.vector.tensor_scalar(
                        out=e[:, 0:nh, cs1:IW].bitcast(i16),
                        in0=ps[:, 0:nh, cs1:IW],
                        scalar1=BL_MUL, scalar2=BL_OFF,
                        op0=mybir.AluOpType.mult,
                        op1=mybir.AluOpType.add)
                st[i][1] = e

            def stage_t(i):
                nonlocal pacc_cur, conv_i
                bi, r, gt, fst, lst, pvf, pvl = work[i]
                qTs, kTs, tbn, vws = tiles[bi]
                e = st[i][1]
                if fst:
                    pacc_cur = ps_acc.tile([128, 4, DV], f32, tag="pacc",
                                           name=f"pacc{bi}_{r}")
                pacc = pacc_cur
                ms = st[i][2]
                for h, t in enumerate(gt):
                    pvlo, mhi = cls[r][t]
                    if mhi > pvlo:
                        if h in ms:
                            nc.gpsimd.tensor_tensor(
                                e[:, h, pvlo:mhi], e[:, h, pvlo:mhi],
                                ms[h][:, 0:mhi - pvlo],
                                op=mybir.AluOpType.mult)
                        else:
                            nc.vector.scalar_tensor_tensor(
                                e[:, h, pvlo:mhi],
                                tbn[:, r * IW + pvlo:r * IW + mhi],
                                jpos[:, t:t + 1],
                                e[:, h, pvlo:mhi],
                                op0=mybir.AluOpType.is_lt,
                                op1=mybir.AluOpType.mult)
                for h, t in enumerate(gt):
                    pvlo = cls[r][t][0]
                    for ib in range(pvlo // 128, 4):
                        po = max(pvlo - ib * 128, 0)
                        nc.tensor.matmul(
                            pacc[po:128, ib, :],
                            e[:, h, ib * 128 + po:(ib + 1) * 128],
                            vws[:, t, :],
                            start=((t, ib) == pvf),
                            stop=((t, ib) == pvl))
                del st[i]
                if lst:
                    o_sb = sb_o.tile([128, 4, DV], f16)
                    if conv_i % 2 == 0:
                        nc.scalar.activation(
                            o_sb, pacc, mybir.ActivationFunctionType.Copy,
                            scale=OSCALE)
                    else:
                        nc.vector.tensor_scalar(
                            out=o_sb, in0=pacc, scalar1=OSCALE, scalar2=None,
                            op0=mybir.AluOpType.mult)
                    conv_i += 1
                    nc.sync.dma_start(out=out[bi, r], in_=o_sb)

            nw = len(work)
            for i in range(nw + d_t):
                if i < nw:
                    stage_s(i)
                if d_e <= i < nw + d_e:
                    stage_e(i - d_e)
                if i >= d_t:
                    stage_t(i - d_t)
    nc.compile()
    return nc


LAST = {}


def kernel(q, k, v, valid, _trace=False):
    q = np.ascontiguousarray(np.asarray(q, dtype=np.float32))
    k = np.ascontiguousarray(np.asarray(k, dtype=np.float32))
    v = np.ascontiguousarray(np.asarray(v, dtype=np.float32))
    t = np.clip(np.asarray(valid).astype(np.int64), 0, N)

    perm = np.argsort(t, axis=1, kind="stable")
    t_s = np.take_along_axis(t, perm, axis=1)
    q_s = np.take_along_axis(q, perm[..., None], axis=1)

    e6 = float(np.exp(np.float32(1e-6)))
    # suffix sums of v in f64: ss[b, tt] = sum_{j >= tt} v[b, j]
    ss = np.zeros((B, N + 1, D), np.float64)
    ss[:, :-1] = np.cumsum(v[:, ::-1, :].astype(np.float64), axis=1)[:, ::-1, :]
    ssg = np.take_along_axis(ss, t_s[..., None], axis=1)       # [B, N, D]
    cnt = (N - t_s).astype(np.float64)                         # [B, N]

    cls = _classify(t_s)
    cut1, mask_eng = _plan(cls)
    if "cuts" in LAST:
        cut1 = LAST["cuts"]
    nc = _build_program(cls, cut1, mask_eng, *LAST.get('depth', (2, 4)))

    # host-packed V with ones column: vw[b, p, tau*DV + d]
    vwh = np.zeros((B, 128, NJ, DV), np.float16)
    vwh[..., 0:D] = v.reshape(B, NJ, 128, D).transpose(0, 2, 1, 3).astype(np.float16)
    vwh[..., D] = 1.0
    vwh = vwh.reshape(B, 128, NJ * DV)

    tbn_h = np.ascontiguousarray(
        (1024 - t_s).astype(np.float16)[:, None, :])

    in_maps = []
    for c in range(NCORES):
        sl = slice(c * NB, (c + 1) * NB)
        in_maps.append({
            "qT": np.ascontiguousarray(
                np.swapaxes(q_s[sl], 1, 2)).astype(np.float16),
            "kT": np.ascontiguousarray(
                np.swapaxes(k[sl], 1, 2)).astype(np.float16),
            "vw": np.ascontiguousarray(vwh[sl]),
            "tbnd": tbn_h[sl],
        })
    res = run_bass_kernel_spmd(nc, in_maps, list(range(NCORES)),
                               trace=_trace)
    LAST["res"] = res
    LAST["nc"] = nc

    # rows the device never wrote; rows using Blinn exp (host bias fix)
    lo_min = [min((cls[r][t][0] for t in range(NJ)
                   if cls[r][t] is not None), default=IW)
              for r in range(NI)]
    written = np.zeros(N, bool)
    blinn = np.zeros(N, bool)
    for r in range(NI):
        written[r * IW + lo_min[r]:(r + 1) * IW] = True
        blinn[r * IW + cut1[r]:(r + 1) * IW] = True
    gamma = np.where(blinn, 1.0 + BL_BIAS, 1.0)     # [N]

    out = np.empty((B, N, D), np.float32)
    for c in range(NCORES):
        o = res.results[c]["out"]                 # [NB, NI, 128, 4*DV] f16
        for bi in range(NB):
            b = c * NB + bi
            ob = o[bi].reshape(NI, 128, 4, DV).transpose(0, 2, 1, 3) \
                      .reshape(N, DV).astype(np.float64) * 16.0
            cn = gamma * e6 * cnt[b]
            cs_ = (gamma * e6)[:, None] * ssg[b]
            num = ob[:, 0:D] + cs_
            den = ob[:, D] + cn
            num[~written] = cs_[~written]
            den[~written] = cn[~written]
            out[b, perm[b]] = (num / den[:, None]).astype(np.float32)
    return out


# revision 35
# speedup vs baseline: 1.0587x; 1.0587x over previous
"""Bass/Trainium2 kernel for masked dot-product attention.

Math (per batch b):
  scores = q @ k^T / sqrt(D)
  masked positions (j >= valid[i]) replaced by 1e-6 (NOT -inf)
  weights = softmax(scores, axis=-1);  out = weights @ v

Strategy:
  - Shard batch dim B=16 across 8 cores (2 batches/core), SPMD program.
  - Host-side: sort rows of each batch by valid[i] (argsort) so the mask is a
    monotone staircase; device computes only the staircase-covered region.
  - Device per (batch, 512-row i-range):
      S^T tiles [j=128, i<=512] on PE (fp16 operands),
      exp split row-consistently between ACT (spline exp, rows < cut) and DVE
      (rows >= cut; one-instruction fp16 "Blinn" bit-trick exp:
      bits = round(s*1024*log2e/8 + offset) written as int16 aliasing the fp16
      e-tile; offset centers the PWL-mantissa log-error at +-3%, and its known
      mean bias is corrected exactly on the host),
      boundary-tile masking via one fused scalar_tensor_tensor (is_lt -> mult)
      load-balanced between DVE and GpSimd,
      PV accumulated per 128-row i-subblock as pacc[i, 0:65] += E_tile^T.T @ V
      (output free dim 65 instead of 512 -> ~2x fewer PE cycles), ones column
      in V gives the softmax denominator for free. PSUM zero regions are one
      whole 2KB bank, so the range's pacc bank gets exactly one start (zeroes
      the bank) and one stop.
  - Out: pacc (PSUM f32) scaled by 1/16 into fp16 SBUF (ACT/DVE alternating),
    DMA'd out on the sync HWDGE ring after all input loads.
  - Host: adds the analytic masked-region correction exp(1e-6)*(suffix sums of
    v) (scaled by 1+BIAS for Blinn rows), divides by the denominator, fills
    never-written rows, unsorts.
"""

import numpy as np

import concourse.bass as bass
import concourse.tile as tile
import concourse.mybir as mybir
from concourse import bacc
from concourse.bass_utils import run_bass_kernel_spmd

B, N, D = 16, 2048, 64
NCORES = 8
NB = B // NCORES          # batches per core
IW = 512                  # i-range width (moving dim of S matmuls)
NI = N // IW              # 4 i-ranges
JW = 128                  # j-tile width (partition dim of S^T)
NJ = N // JW              # 16 j-tiles
DV = D + 1                # V with ones column appended
G = 2                     # j-tiles per exp group (PSUM: 3*2 + 2 = 8 banks)

f32 = mybir.dt.float32
f16 = mybir.dt.float16
i16 = mybir.dt.int16
i32 = mybir.dt.int32

SCALE = 0.125             # 1/sqrt(D)
# Blinn fp16 exp: bits = round(s*BL_MUL + BL_OFF) viewed as fp16 ~= e^(s/8).
# g(f) = log2(1+f) - f in [0, 0.08607]; centering at c = 0.04304 makes the
# multiplicative noise +-3.03% with mean bias E[2^(g-c)] = 1 + BL_BIAS that
# the host corrects exactly.
_C_CENTER = 0.0430374
BL_MUL = float(1024.0 * SCALE / np.log(2.0))
BL_OFF = float(15360.0 - 1024.0 * _C_CENTER)
_f = np.linspace(0.0, 1.0, 200001)
BL_BIAS = float(np.trapezoid((1.0 + _f) * 2.0 ** (-_f), _f) * 2.0 ** (-_C_CENTER) - 1.0)
OSCALE = 1.0 / 16.0       # pacc -> fp16 out scaling (overflow headroom)

# cost-model constants for the ACT/DVE/Pool balance (ns per element / instr)
_ACT_NS = 0.8333
_DVE_NS = 1.0417
_POOL_NS = 0.8333 / 0.6
_ACT_OH = 185.0
_DVE_OH = 125.0
_POOL_OH = 160.0


def _round_pv(alo):
    m = alo % 128
    return (alo // 128) * 128 + (64 if m >= 64 else (32 if m >= 32 else 0))


def _classify(t_sorted):
    """Per (i-range, j-tile): None (skip) or (pvlo, mhi): compute columns
    [pvlo, IW) (pvlo rounded down so partition bases land on {0,32,64});
    the fused mask zeroes [pvlo, mhi)."""
    cls = []
    for r in range(NI):
        row = []
        tw = t_sorted[:, r * IW:(r + 1) * IW]  # [B, IW] sorted ascending
        for tau in range(NJ):
            jlo, jhi = JW * tau, JW * (tau + 1)
            n_le = (tw <= jlo).sum(axis=1)   # rows fully masked in this tile
            n_lt = (tw < jhi).sum(axis=1)    # rows with any mask in this tile
            if (n_le == IW).all():
                row.append(None)
            else:
                alo = int(n_le.min())
                pvlo = _round_pv(alo)
                mhi = max(int(n_lt.max()), alo)
                row.append((pvlo, max(mhi, pvlo)))
        cls.append(row)
    return cls


def _groups(cls, r):
    taus = [t for t in range(NJ) if cls[r][t] is not None]
    return taus, [taus[i:i + G] for i in range(0, len(taus), G)]


def _plan(cls):
    """Numerically balance ACT / DVE / Pool exp shares.

    Rows [plo, cut1) -> ACT spline exp; [cut1, cut2) -> DVE Blinn;
    [cut2, IW) -> Pool Blinn. Returns (cut1, cut2) per range."""
    ranges = []
    masks = []
    conv_n = 0
    for r in range(NI):
        taus, groups = _groups(cls, r)
        if taus:
            conv_n += 1
        plos = [min(cls[r][t][0] for t in gt) for gt in groups]
        nhs = [len(gt) for gt in groups]
        ranges.append((plos, nhs))
        for t in taus:
            pvlo, mhi = cls[r][t]
            if mhi > pvlo:
                masks.append((r, t, mhi - pvlo))
    conv_n *= NB

    def predict(f_act, g_pool):
        cut1 = []
        act = dve = pool = 0.0
        for plos, nhs in ranges:
            if not plos:
                cut1.append(IW)
                continue
            area = sum((IW - p) * n for p, n in zip(plos, nhs))

            def pick(frac_above):
                best, bestd = IW, abs(frac_above * area)
                for c in range(0, IW + 1, 16):
                    above = sum((IW - max(c, p)) * n for p, n in zip(plos, nhs))
                    dd = abs(above - frac_above * area)
                    if dd < bestd:
                        best, bestd = c, dd
                return best

            c1 = pick(1.0 - f_act)
            cut1.append(c1)
            for p, n in zip(plos, nhs):
                aw = max(0, min(c1, IW) - p)
                dw = IW - max(c1, p)
                if aw > 0:
                    act += (_ACT_NS * aw * n + _ACT_OH) * NB
                if dw > 0:
                    dve += (_DVE_NS * dw * n + _DVE_OH) * NB
        # offload the mult half of the g_pool largest mask columns to Pool
        eng = {}
        order = sorted(masks, key=lambda m: -m[2])
        target = sum(m[2] for m in masks) * g_pool
        acc = 0.0
        for r_, t_, w in order:
            if acc < target:
                eng[(r_, t_)] = 'p'
                dve += (0.52 * w + _DVE_OH) * NB          # is_lt at 2x
                pool += (2.0 * w + 255.0) * NB            # mult, eff 0.42
                acc += w
            else:
                eng[(r_, t_)] = 'd'
                dve += (_DVE_NS * w + _DVE_OH) * NB       # fused STT
        act += (conv_n // 2 + conv_n % 2) * (_ACT_NS * 4 * DV + _ACT_OH)
        dve += (conv_n // 2) * (_DVE_NS * 4 * DV + _DVE_OH)
        act += 1283.0   # act table load
        return act, dve, pool, cut1, eng

    # knobs fitted against TimelineSim sweeps (predict() underestimates the
    # critical-path effects, so the analytic argmin is not used directly)
    _, _, _, cut1, eng = predict(0.63, 0.4)
    return cut1, eng


LAST_BUILD_OPTS = {}


def _build_program(cls, cut1, mask_eng, d_e=2, d_t=4):
    nc = bacc.Bacc("TRN2", target_bir_lowering=False, debug=False)

    qT = nc.dram_tensor("qT", [NB, D, N], f16, kind="ExternalInput").ap()
    kT = nc.dram_tensor("kT", [NB, D, N], f16, kind="ExternalInput").ap()
    vw = nc.dram_tensor("vw", [NB, 128, NJ * DV], f16, kind="ExternalInput").ap()
    tbnd = nc.dram_tensor("tbnd", [NB, 128, N], f16, kind="ExternalInput").ap()
    out = nc.dram_tensor("out", [NB, NI, 128, 4 * DV], f16,
                         kind="ExternalOutput").ap()

    H = N // 2

    with tile.TileContext(nc, trace_sim=False) as tc:
        with (
            tc.tile_pool(name="consts", bufs=1) as consts,
            tc.tile_pool(name="sb_T", bufs=2) as sb_T,
            tc.tile_pool(name="sb_v", bufs=2) as sb_v,
            tc.tile_pool(name="sb_e", bufs=6) as sb_e,
            tc.tile_pool(name="sb_o", bufs=3) as sb_o,
            tc.tile_pool(name="sb_m", bufs=6) as sb_m,
            tc.tile_pool(name="ps_s", bufs=3, space="PSUM") as ps_s,
            tc.tile_pool(name="ps_acc", bufs=2, space="PSUM") as ps_acc,
        ):
            jpos_i = consts.tile([128, NJ], i32)
            nc.gpsimd.iota(jpos_i, pattern=[[-JW, NJ]], base=1024,
                           channel_multiplier=-1)
            jpos = consts.tile([128, NJ], f32)
            nc.vector.tensor_copy(jpos, jpos_i)

            tiles = []
            for bi in range(NB):
                qTs = sb_T.tile([D, N], f16, tag="qT", name=f"qT{bi}")
                kTs = sb_T.tile([D, N], f16, tag="kT", name=f"kT{bi}")
                tbn = sb_T.tile([128, N], f16, tag="tbn", name=f"tbn{bi}")
                vws = sb_v.tile([128, NJ, DV], f16, tag="vw", name=f"vw{bi}")
                # need-ordered chunked loads on three HWDGE rings
                nc.sync.dma_start(out=kTs[:, 0:H], in_=kT[bi][:, 0:H])
                nc.scalar.dma_start(out=qTs[:, 0:H], in_=qT[bi][:, 0:H])
                nc.sync.dma_start(out=kTs[:, H:N], in_=kT[bi][:, H:N])
                nc.scalar.dma_start(out=qTs[:, H:N], in_=qT[bi][:, H:N])
                nc.sync.dma_start(out=vws, in_=vw[bi])
                nc.gpsimd.dma_start(out=tbn[:, 0:H], in_=tbnd[bi][:, 0:H])
                nc.gpsimd.dma_start(out=tbn[:, H:N], in_=tbnd[bi][:, H:N])
                tiles.append((qTs, kTs, tbn, vws))

            # flatten all (batch, range, group) work; software-pipeline three
            # stages (S at i+2, exp at i+1, masks+PV at i) so waiting PV
            # matmuls never head-of-line-block the next S on the PE queue.
            per_bi = []
            for bi in range(NB):
                wl = []
                rs = range(NI) if bi == 0 else range(NI - 1, -1, -1)
                for r in rs:
                    taus, groups = _groups(cls, r)
                    if not taus:
                        continue
                    pv_seq = [(t, ib) for t in taus
                              for ib in range(cls[r][t][0] // 128, 4)]
                    for g, gt in enumerate(groups):
                        wl.append((bi, r, gt, g == 0, g == len(groups) - 1,
                                   pv_seq[0], pv_seq[-1]))
                per_bi.append(wl)
            if LAST_BUILD_OPTS.get("interleave"):
                work = []
                a, b = per_bi
                for i in range(max(len(a), len(b))):
                    if i < len(a):
                        work.append(a[i])
                    if i < len(b):
                        work.append(b[i])
            else:
                work = per_bi[0] + per_bi[1]

            conv_i = 0
            st = {}      # per in-flight index: (ps, e, pacc_of_range)
            pacc_cur = None

            def stage_s(i):
                nonlocal pacc_cur
                bi, r, gt, fst, lst, pvf, pvl = work[i]
                qTs, kTs, tbn, vws = tiles[bi]
                nh = len(gt)
                ps = ps_s.tile([128, G, IW], f32)
                for h, t in enumerate(gt):
                    pvlo = cls[r][t][0]
                    nc.tensor.matmul(
                        ps[:, h, pvlo:IW],
                        kTs[:, bass.ts(t, JW)],
                        qTs[:, r * IW + pvlo:(r + 1) * IW],
                        start=True, stop=True)
                ms = {}
                for h, t in enumerate(gt):
                    pvlo, mhi = cls[r][t]
                    if mhi > pvlo and mask_eng.get((r, t)) == 'p':
                        m = sb_m.tile([128, IW], f16, tag="m",
                                      name=f"m{i}_{h}")
                        nc.vector.tensor_scalar(
                            out=m[:, 0:mhi - pvlo],
                            in0=tbn[:, r * IW + pvlo:r * IW + mhi],
                            scalar1=jpos[:, t:t + 1], scalar2=None,
                            op0=mybir.AluOpType.is_lt)
                        ms[h] = m
                st[i] = [ps, None, ms]

            def stage_e(i):
                bi, r, gt, fst, lst, pvf, pvl = work[i]
                nh = len(gt)
                ps = st[i][0]
                e = sb_e.tile([128, G, IW], f16)
                plo = min(cls[r][t][0] for t in gt)
                cs1 = min(max(cut1[r], plo), IW)
                if cs1 > plo:
                    nc.scalar.activation(
                        e[:, 0:nh, plo:cs1], ps[:, 0:nh, plo:cs1],
                        mybir.ActivationFunctionType.Exp, scale=SCALE)
                if cs1 < IW:
                    nc.vector.tensor_scalar(
                        out=e[:, 0:nh, cs1:IW].bitcast(i16),
                        in0=ps[:, 0:nh, cs1:IW],
                        scalar1=BL_MUL, scalar2=BL_OFF,
                        op0=mybir.AluOpType.mult,
                        op1=mybir.AluOpType.add)
                st[i][1] = e

            def stage_t(i):
                nonlocal pacc_cur, conv_i
                bi, r, gt, fst, lst, pvf, pvl = work[i]
                qTs, kTs, tbn, vws = tiles[bi]
                e = st[i][1]
                if fst:
                    pacc_cur = ps_acc.tile([128, 4, DV], f32, tag="pacc",
                                           name=f"pacc{bi}_{r}")
                pacc = pacc_cur
                ms = st[i][2]
                for h, t in enumerate(gt):
                    pvlo, mhi = cls[r][t]
                    if mhi > pvlo:
                        if h in ms:
                            nc.gpsimd.tensor_tensor(
                                e[:, h, pvlo:mhi], e[:, h, pvlo:mhi],
                                ms[h][:, 0:mhi - pvlo],
                                op=mybir.AluOpType.mult)
                        else:
                            nc.vector.scalar_tensor_tensor(
                                e[:, h, pvlo:mhi],
                                tbn[:, r * IW + pvlo:r * IW + mhi],
                                jpos[:, t:t + 1],
                                e[:, h, pvlo:mhi],
                                op0=mybir.AluOpType.is_lt,
                                op1=mybir.AluOpType.mult)
                for h, t in enumerate(gt):
                    pvlo = cls[r][t][0]
                    for ib in range(pvlo // 128, 4):
                        po = max(pvlo - ib * 128, 0)
                        nc.tensor.matmul(
                            pacc[po:128, ib, :],
                            e[:, h, ib * 128 + po:(ib + 1) * 128],
                            vws[:, t, :],
                            start=((t, ib) == pvf),
                            stop=((t, ib) == pvl))
                del st[i]
                if lst:
                    o_sb = sb_o.tile([128, 4, DV], f16)
                    if conv_i % 2 == 0:
                        nc.scalar.activation(
                            o_sb, pacc, mybir.ActivationFunctionType.Copy,
                            scale=OSCALE)
                    else:
                        nc.vector.tensor_scalar(
                            out=o_sb, in0=pacc, scalar1=OSCALE, scalar2=None,
                            op0=mybir.AluOpType.mult)
                    conv_i += 1
                    nc.sync.dma_start(out=out[bi, r], in_=o_sb)

            nw = len(work)
            for i in range(nw + d_t):
                if i < nw:
                    stage_s(i)
                if d_e <= i < nw + d_e:
                    stage_e(i - d_e)
                if i >= d_t:
                    stage_t(i - d_t)
    nc.compile()
    return nc


LAST = {}


def kernel(q, k, v, valid, _trace=False):
    q = np.ascontiguousarray(np.asarray(q, dtype=np.float32))
    k = np.ascontiguousarray(np.asarray(k, dtype=np.float32))
    v = np.ascontiguousarray(np.asarray(v, dtype=np.float32))
    t = np.clip(np.asarray(valid).astype(np.int64), 0, N)

    perm = np.argsort(t, axis=1, kind="stable")
    t_s = np.take_along_axis(t, perm, axis=1)
    q_s = np.take_along_axis(q, perm[..., None], axis=1)

    e6 = float(np.exp(np.float32(1e-6)))
    # suffix sums of v in f64: ss[b, tt] = sum_{j >= tt} v[b, j]
    ss = np.zeros((B, N + 1, D), np.float64)
    ss[:, :-1] = np.cumsum(v[:, ::-1, :].astype(np.float64), axis=1)[:, ::-1, :]
    ssg = np.take_along_axis(ss, t_s[..., None], axis=1)       # [B, N, D]
    cnt = (N - t_s).astype(np.float64)                         # [B, N]

    cls = _classify(t_s)
    cut1, mask_eng = _plan(cls)
    if "cuts" in LAST:
        cut1 = LAST["cuts"]
    nc = _build_program(cls, cut1, mask_eng, *LAST.get('depth', (2, 4)))

    # host-packed V with ones column: vw[b, p, tau*DV + d]
    vwh = np.zeros((B, 128, NJ, DV), np.float16)
    vwh[..., 0:D] = v.reshape(B, NJ, 128, D).transpose(0, 2, 1, 3).astype(np.float16)
    vwh[..., D] = 1.0
    vwh = vwh.reshape(B, 128, NJ * DV)

    tbn_h = np.ascontiguousarray(np.broadcast_to(
        (1024 - t_s).astype(np.float16)[:, None, :], (B, 128, N)))

    in_maps = []
    for c in range(NCORES):
        sl = slice(c * NB, (c + 1) * NB)
        in_maps.append({
            "qT": np.ascontiguousarray(
                np.swapaxes(q_s[sl], 1, 2)).astype(np.float16),
            "kT": np.ascontiguousarray(
                np.swapaxes(k[sl], 1, 2)).astype(np.float16),
            "vw": np.ascontiguousarray(vwh[sl]),
            "tbnd": tbn_h[sl],
        })
    res = run_bass_kernel_spmd(nc, in_maps, list(range(NCORES)),
                               trace=_trace)
    LAST["res"] = res
    LAST["nc"] = nc

    # rows the device never wrote; rows using Blinn exp (host bias fix)
    lo_min = [min((cls[r][t][0] for t in range(NJ)
                   if cls[r][t] is not None), default=IW)
              for r in range(NI)]
    written = np.zeros(N, bool)
    blinn = np.zeros(N, bool)
    for r in range(NI):
        written[r * IW + lo_min[r]:(r + 1) * IW] = True
        blinn[r * IW + cut1[r]:(r + 1) * IW] = True
    gamma = np.where(blinn, 1.0 + BL_BIAS, 1.0)     # [N]

    out = np.empty((B, N, D), np.float32)
    for c in range(NCORES):
        o = res.results[c]["out"]                 # [NB, NI, 128, 4*DV] f16
        for bi in range(NB):
            b = c * NB + bi
            ob = o[bi].reshape(NI, 128, 4, DV).transpose(0, 2, 1, 3) \
                      .reshape(N, DV).astype(np.float64) * 16.0
            cn = gamma * e6 * cnt[b]
            cs_ = (gamma * e6)[:, None] * ssg[b]
            num = ob[:, 0:D] + cs_
            den = ob[:, D] + cn
            num[~written] = cs_[~written]
            den[~written] = cn[~written]
            out[b, perm[b]] = (num / den[:, None]).astype(np.float32)
    return out
